# revision 11
# baseline (speedup 1.0000x reference)
"""CRF forward (log-partition) kernel for Trainium2, 8 NeuronCores.

Algorithm: the forward recurrence in rescaled linear space is
    p_{t+1} = diag(exp(u_t)) @ E @ p_t,   E = exp(transitions)
(contraction over j = second index of transitions). Products of positive
matrices are strong Hilbert-projective contractions (~2 orders of
magnitude per step here), so the normalized direction of p_t forgets its
initial condition within a handful of steps. This cuts the T=65536-step
chain into C=2048 chunks of L=32 steps, each re-derived independently
from an arbitrary start vector with a W=6-step warm-up halo, with NO
cross-chunk communication:

    logZ = sum_chunks sum_owned ln(n_s) + SHIFT*T + ln(tau . qhat_T)

Per core: 256 chunks as TWO independent interleaved streams of B=128
chunk-columns (stream Y phase-shifted half a round behind X) so the
per-step dependency chain of one stream (matmul -> fused mul+norm ->
transpose -> PSUM evict) hides under the other stream's engine work.

Per stream step:
  - 8 DoubleRow fp8(e4m3) matmuls (K=256 each) compute psum[b,i] =
    sum_j q[j,b] * E^T[j,i] (stationary = state jt-pair, each LDWEIGHTS
    feeds the two 512-column halves).
  - one DVE scalar_tensor_tensor: qhat = (psum * 1/n_prev) * exp(u_row)
    with fused free-dim sum -> n_s (the per-column L1 normalizer,
    recorded in nbuf; exact per-step normalization keeps the fp8 state
    in range). exp(u) is batch-precomputed on ScalarE per DMA group
    with u pre-shifted by -(8 - ln 32) on host so n_s stays ~32.
  - 8 transpose-matmuls against an fp8 identity (2 groups of 4) +
    ScalarE PSUM->SBUF evictions produce the next stationary [j,b] pairs.

ln(n) slots are batch-computed at the end (one Ln + reduce per stream).
Chunk 0 of core 0 uses fake unary rows (0 at start_idx, -14 else)
which force q to the exact one-hot start vector within a few steps.
"""

import math
import numpy as np
import ml_dtypes
from contextlib import ExitStack

T = 65536
N = 1024
NCORES = 8
B = 128           # chunk-columns per stream (matmul M dim)
NSTREAM = 2
L = 32            # chunk length (owned steps)
W = 4             # warm-up halo steps
STEPS = W + L     # 36
PERCORE = T // NCORES          # 8192
PERSTREAM = PERCORE // NSTREAM  # 4096
SHIFT = float(np.float32(8.0 - math.log(32.0)))  # u preshift; added back on host

USE_R8 = False    # error-feedback residual matmul pass (accuracy fallback)

_BF = ml_dtypes.bfloat16
_F8 = ml_dtypes.float8_e4m3

_compiled = {}


def _build_bass():
    import concourse.bacc as bacc
    import concourse.tile as tile
    from concourse import mybir
    from concourse.masks import make_identity

    bf = mybir.dt.bfloat16
    f8 = mybir.dt.float8e4
    f32 = mybir.dt.float32
    AF = mybir.ActivationFunctionType
    ALU = mybir.AluOpType
    DR = mybir.MatmulPerfMode.DoubleRowSwInterleave

    nc = bacc.Bacc("TRN2", name="crf_fwd2")

    U = nc.dram_tensor("u", [NSTREAM, B, STEPS, N], bf, kind="ExternalInput")
    ET8 = nc.dram_tensor("et", [N, N], f8, kind="ExternalInput")
    if USE_R8:
        ETR = nc.dram_tensor("etr", [N, N], f8, kind="ExternalInput")
    OUT_C = nc.dram_tensor("csum", [B, NSTREAM], f32, kind="ExternalOutput")
    OUT_Q = nc.dram_tensor("qfin", [NSTREAM, B, N], f32, kind="ExternalOutput")

    with tile.TileContext(nc) as tc, ExitStack() as ctx:
        consts = ctx.enter_context(tc.tile_pool(name="consts", bufs=1))
        upool = ctx.enter_context(tc.tile_pool(name="u", bufs=3))
        qpool = ctx.enter_context(tc.tile_pool(name="qhat", bufs=3))
        qnpool = ctx.enter_context(tc.tile_pool(name="qn", bufs=4))
        smalls = ctx.enter_context(tc.tile_pool(name="smalls", bufs=6))
        ps_mm = ctx.enter_context(tc.tile_pool(name="psmm", bufs=2, space="PSUM"))
        ps_tr = ctx.enter_context(tc.tile_pool(name="pstr", bufs=2, space="PSUM"))

        ident = consts.tile([128, 128], f8)
        make_identity(nc, ident)

        # et_sb[j, jt, i] = E^T[jt*128+j, i] = E[i, jt*128+j]
        et_sb = consts.tile([128, 8, N], f8)
        nc.sync.dma_start(out=et_sb[:], in_=ET8.ap().rearrange("(jt j) i -> j jt i", j=128))
        if USE_R8:
            etr_sb = consts.tile([128, 8, N], f8)
            nc.sync.dma_start(
                out=etr_sb[:], in_=ETR.ap().rearrange("(jt j) i -> j jt i", j=128)
            )

        # per-(column, step) L1 normalizers, one slot each (Ln'd at the end)
        nbuf = consts.tile([128, NSTREAM, STEPS], f32)
        nc.vector.memset(nbuf[:], 1.0)

        # initial state: uniform ones, layout [j_in_tile, jt, b]
        q_init = consts.tile([128, 4, 128, 2], f8)
        nc.vector.memset(q_init[:], 1.0 / 32.0)
        rm_init = consts.tile([128, NSTREAM], f32)
        nc.vector.memset(rm_init[:], 1.0)

        st = [
            {"qcur": q_init, "rm": rm_init[:, s : s + 1], "eu": None,
             "psum": None, "qhat": None, "nlast": None}
            for s in range(NSTREAM)
        ]

        def front(S, r):
            s = st[S]
            if r % 4 == 0:
                gl = min(4, STEPS - r)
                ut = upool.tile([128, gl, N], bf, tag=f"u{S}")
                nc.sync.dma_start(out=ut[:], in_=U[S, :, r : r + gl, :])
                s["ut"] = ut
            s["eu"] = s["ut"][:, r % 4, :]
            psum = ps_mm.tile([128, N], f32, tag="mm")
            q = s["qcur"]
            npass = 2 if USE_R8 else 1
            for p in range(4):
                lhsT = q[:, p, :, :].opt({0})
                for h in range(2):
                    nc.tensor.matmul(
                        psum[:, h * 512 : (h + 1) * 512],
                        lhsT,
                        et_sb[:, 2 * p : 2 * p + 2, h * 512 : (h + 1) * 512],
                        start=(p == 0),
                        stop=(p == 3 and npass == 1),
                        perf_mode=DR,
                        skip_group_check=True,
                    )
            if USE_R8:
                for p in range(4):
                    for h in range(2):
                        nc.tensor.matmul(
                            psum[:, h * 512 : (h + 1) * 512],
                            q[:, 2 * p : 2 * p + 2, :],
                            etr_sb[:, 2 * p : 2 * p + 2, h * 512 : (h + 1) * 512],
                            start=False,
                            stop=(p == 3),
                            perf_mode=DR,
                            skip_group_check=True,
                        )
            s["psum"] = psum

        def back(S, r):
            s = st[S]
            eu = s["eu"]
            qhat = qpool.tile([128, N], f8, tag=f"qh{S}")
            n_ap = nbuf[:, S, r : r + 1]
            nh = smalls.tile([128, 2], f32, tag=f"nh{S}")
            for h in range(2):
                cs = slice(h * 512, (h + 1) * 512)
                nc.vector.scalar_tensor_tensor(
                    out=qhat[:, cs],
                    in0=s["psum"][:, cs],
                    scalar=s["rm"],
                    in1=eu[:, cs],
                    op0=ALU.mult,
                    op1=ALU.mult,
                    accum_out=nh[:, h : h + 1],
                )
            nc.vector.tensor_add(n_ap, nh[:, 0:1], nh[:, 1:2])
            rm = smalls.tile([128, 1], f32, tag=f"rm{S}")
            nc.vector.reciprocal(rm[:], n_ap)
            s["rm"] = rm[:]
            s["qhat"] = qhat
            if r == STEPS - 1:
                s["nlast"] = n_ap
                return
            qn = qnpool.tile([128, 4, 128, 2], f8, tag=f"qn{S}")
            for grp in range(2):
                tr = ps_tr.tile([128, 2, 2, 128], f32, tag="tr")
                for i4 in range(4):
                    it = grp * 4 + i4
                    nc.tensor.matmul(
                        tr[:, i4 // 2, i4 % 2, :],
                        qhat[:, it * 128 : (it + 1) * 128],
                        ident[:],
                        start=True,
                        stop=True,
                    )
                out_ap = qn[:, 2 * grp : 2 * grp + 2, ::-1, :].transpose([0, 1, 3, 2])
                nc.scalar.activation(out_ap, tr[:], AF.Copy)
            s["qcur"] = qn

        for r in range(STEPS):
            front(0, r)
            if r > 0:
                back(1, r - 1)
            front(1, r)
            back(0, r)
        back(1, STEPS - 1)

        # epilogue: csum = sum of ln(n) over owned steps; qfin = qhat/n
        for S in range(NSTREAM):
            lnt = consts.tile([128, L], f32, tag=f"ln{S}")
            nc.scalar.activation(lnt[:], nbuf[:, S, W:STEPS], AF.Ln)
            cs = smalls.tile([128, 1], f32, tag=f"cs{S}")
            nc.vector.tensor_reduce(cs[:], lnt[:], axis=mybir.AxisListType.X, op=ALU.add)
            nc.sync.dma_start(out=OUT_C[:, S : S + 1], in_=cs[:])

            rn = smalls.tile([128, 1], f32, tag=f"rn{S}")
            nc.vector.reciprocal(rn[:], st[S]["nlast"])
            qf = consts.tile([128, N], f32, tag=f"qf{S}")
            nc.scalar.activation(qf[:], st[S]["qhat"][:], AF.Copy, scale=rn[:])
            nc.sync.dma_start(out=OUT_Q[S, :, :], in_=qf[:])

    nc.finalize()
    _dedupe_ldweights(nc)
    return nc


def _dedupe_ldweights(nc):
    """Remove back-to-back duplicate Ldweights (same memref/offset/ap/perf_mode
    separated only by Matmults). bacc emits one Ldweights per matmul; our
    DoubleRow pairs issue two matmuls (the two 512-col PSUM halves) from the
    same stationary, so every second load is redundant (~213 ns each on the
    PE weight path). Only sync-free duplicates are dropped, so the semaphore
    graph is unchanged."""
    removed = 0
    for fn in nc.m.functions:
        for blk in fn.blocks:
            out = []
            last_key = None
            for ins in blk.instructions:
                if ins.opcode == "Ldweights":
                    w = ins.ins[0]
                    key = (w.memref, w.offset, str(w.ap),
                           str(getattr(ins, "perf_mode", None)))
                    si = ins.sync_info
                    clean = si is None or (not si.on_wait and not si.on_update)
                    if clean and key == last_key:
                        removed += 1
                        continue
                    last_key = key
                elif ins.opcode != "Matmult":
                    last_key = None
                out.append(ins)
            blk.instructions = out
    return removed


def _get_nc():
    if "nc" not in _compiled:
        _compiled["nc"] = _build_bass()
    return _compiled["nc"]


def _prep_inputs(unary, transitions, start_idx):
    """Host-side: bf16 cast + per-core/stream halo gather into [NSTREAM, B, STEPS, N]."""
    unary = np.asarray(unary, dtype=np.float32)
    transitions = np.asarray(transitions, dtype=np.float32)

    fake = np.full((W, N), -14.0, dtype=np.float32)
    fake[:, start_idx] = 0.0
    g = np.exp(np.concatenate([fake, unary - np.float32(SHIFT)], axis=0)).astype(_BF)

    e32 = np.exp(transitions).T  # [j, i] = E[i, j]
    et8 = np.ascontiguousarray(e32).astype(_F8)
    maps_extra = {}
    if USE_R8:
        etr = (e32 - et8.astype(np.float32)).astype(_F8)
        maps_extra["etr"] = etr

    row_bytes = N * 2
    in_maps = []
    for c in range(NCORES):
        views = []
        for S in range(NSTREAM):
            base = g[c * PERCORE + S * PERSTREAM :]
            v = np.lib.stride_tricks.as_strided(
                base, shape=(B, STEPS, N), strides=(L * row_bytes, row_bytes, 2)
            )
            views.append(v)
        u_c = np.ascontiguousarray(np.stack(views, axis=0))
        in_maps.append({"u": u_c, "et": et8, **maps_extra})
    return in_maps


def _combine(results, transitions, end_idx):
    transitions = np.asarray(transitions, dtype=np.float64)
    total = 0.0
    for r in results:
        total += float(r["csum"].astype(np.float64).sum())
    total += SHIFT * T
    q_T = results[-1]["qfin"][NSTREAM - 1, B - 1].astype(np.float64)
    tau = np.exp(transitions[end_idx])
    total += float(np.log(np.dot(tau, q_T)))
    return total


def kernel(unary, transitions, start_idx, end_idx, _trace=False):
    from concourse.bass_utils import run_bass_kernel_spmd

    start_idx = int(np.asarray(start_idx))
    end_idx = int(np.asarray(end_idx))

    nc = _get_nc()
    in_maps = _prep_inputs(unary, transitions, start_idx)
    res = run_bass_kernel_spmd(nc, in_maps, core_ids=list(range(NCORES)), trace=_trace)
    _compiled["last_result"] = res
    logZ = _combine(res.results, transitions, end_idx)
    return np.array(logZ, dtype=np.float32)


# revision 13
# speedup vs baseline: 1.0472x; 1.0472x over previous
"""CRF forward (log-partition) kernel for Trainium2, 8 NeuronCores.

Algorithm: the forward recurrence in rescaled linear space is
    p_{t+1} = diag(exp(u_t)) @ E @ p_t,   E = exp(transitions)
(contraction over j = second index of transitions). Products of positive
matrices are strong Hilbert-projective contractions (~2 orders of
magnitude per step here), so the normalized direction of p_t forgets its
initial condition within a handful of steps. This cuts the T=65536-step
chain into C=2048 chunks of L=32 steps, each re-derived independently
from an arbitrary start vector with a W=6-step warm-up halo, with NO
cross-chunk communication:

    logZ = sum_chunks sum_owned ln(n_s) + SHIFT*T + ln(tau . qhat_T)

Per core: 256 chunks as TWO independent interleaved streams of B=128
chunk-columns (stream Y phase-shifted half a round behind X) so the
per-step dependency chain of one stream (matmul -> fused mul+norm ->
transpose -> PSUM evict) hides under the other stream's engine work.

Per stream step:
  - 8 DoubleRow fp8(e4m3) matmuls (K=256 each) compute psum[b,i] =
    sum_j q[j,b] * E^T[j,i] (stationary = state jt-pair, each LDWEIGHTS
    feeds the two 512-column halves).
  - one DVE scalar_tensor_tensor: qhat = (psum * 1/n_prev) * exp(u_row)
    with fused free-dim sum -> n_s (the per-column L1 normalizer,
    recorded in nbuf; exact per-step normalization keeps the fp8 state
    in range). exp(u) is batch-precomputed on ScalarE per DMA group
    with u pre-shifted by -(8 - ln 32) on host so n_s stays ~32.
  - 8 transpose-matmuls against an fp8 identity (2 groups of 4) +
    ScalarE PSUM->SBUF evictions produce the next stationary [j,b] pairs.

ln(n) slots are batch-computed at the end (one Ln + reduce per stream).
Chunk 0 of core 0 uses fake unary rows (0 at start_idx, -14 else)
which force q to the exact one-hot start vector within a few steps.
"""

import math
import numpy as np
import ml_dtypes
from contextlib import ExitStack

T = 65536
N = 1024
NCORES = 8
B = 128           # chunk-columns per stream (matmul M dim)
NSTREAM = 2
L = 32            # chunk length (owned steps)
W = 3             # warm-up halo steps
STEPS = W + L     # 35
PERCORE = T // NCORES          # 8192
PERSTREAM = PERCORE // NSTREAM  # 4096
SHIFT = float(np.float32(8.0 - math.log(32.0)))  # u preshift; added back on host

USE_R8 = False    # error-feedback residual matmul pass (accuracy fallback)

_BF = ml_dtypes.bfloat16
_F8 = ml_dtypes.float8_e4m3

_compiled = {}


def _build_bass():
    import concourse.bacc as bacc
    import concourse.tile as tile
    from concourse import mybir
    from concourse.masks import make_identity

    bf = mybir.dt.bfloat16
    f8 = mybir.dt.float8e4
    f32 = mybir.dt.float32
    AF = mybir.ActivationFunctionType
    ALU = mybir.AluOpType
    DR = mybir.MatmulPerfMode.DoubleRow

    nc = bacc.Bacc("TRN2", name="crf_fwd2")

    U = nc.dram_tensor("u", [NSTREAM, B, STEPS, N], bf, kind="ExternalInput")
    ET8 = nc.dram_tensor("et", [N, N], f8, kind="ExternalInput")
    if USE_R8:
        ETR = nc.dram_tensor("etr", [N, N], f8, kind="ExternalInput")
    OUT_C = nc.dram_tensor("csum", [B, NSTREAM], f32, kind="ExternalOutput")
    OUT_Q = nc.dram_tensor("qfin", [NSTREAM, B, N], f32, kind="ExternalOutput")

    with tile.TileContext(nc) as tc, ExitStack() as ctx:
        consts = ctx.enter_context(tc.tile_pool(name="consts", bufs=1))
        upool = ctx.enter_context(tc.tile_pool(name="u", bufs=3))
        qpool = ctx.enter_context(tc.tile_pool(name="qhat", bufs=3))
        qnpool = ctx.enter_context(tc.tile_pool(name="qn", bufs=4))
        smalls = ctx.enter_context(tc.tile_pool(name="smalls", bufs=6))
        ps_mm = ctx.enter_context(tc.tile_pool(name="psmm", bufs=2, space="PSUM"))
        ps_tr = ctx.enter_context(tc.tile_pool(name="pstr", bufs=2, space="PSUM"))

        ident = consts.tile([128, 128], f8)
        make_identity(nc, ident)

        # et_sb[j, jt, i] = E^T[jt*128+j, i] = E[i, jt*128+j]
        et_sb = consts.tile([128, 8, N], f8)
        nc.sync.dma_start(out=et_sb[:], in_=ET8.ap().rearrange("(jt j) i -> j jt i", j=128))
        if USE_R8:
            etr_sb = consts.tile([128, 8, N], f8)
            nc.sync.dma_start(
                out=etr_sb[:], in_=ETR.ap().rearrange("(jt j) i -> j jt i", j=128)
            )

        # per-(column, step) L1 normalizers, one slot each (Ln'd at the end)
        nbuf = consts.tile([128, NSTREAM, STEPS], f32)
        nc.vector.memset(nbuf[:], 1.0)

        # initial state: uniform ones, layout [j_in_tile, jt, b]
        q_init = consts.tile([128, 8, B], f8)
        nc.vector.memset(q_init[:], 1.0 / 32.0)
        rm_init = consts.tile([128, NSTREAM], f32)
        nc.vector.memset(rm_init[:], 1.0)

        st = [
            {"qcur": q_init, "rm": rm_init[:, s : s + 1], "eu": None,
             "psum": None, "qhat": None, "nlast": None}
            for s in range(NSTREAM)
        ]

        def front(S, r):
            s = st[S]
            if r % 4 == 0:
                gl = min(4, STEPS - r)
                ut = upool.tile([128, gl, N], bf, tag=f"u{S}")
                nc.sync.dma_start(out=ut[:], in_=U[S, :, r : r + gl, :])
                s["ut"] = ut
            s["eu"] = s["ut"][:, r % 4, :]
            psum = ps_mm.tile([128, N], f32, tag="mm")
            q = s["qcur"]
            npass = 2 if USE_R8 else 1
            for p in range(4):
                for h in range(2):
                    nc.tensor.matmul(
                        psum[:, h * 512 : (h + 1) * 512],
                        q[:, 2 * p : 2 * p + 2, :],
                        et_sb[:, 2 * p : 2 * p + 2, h * 512 : (h + 1) * 512],
                        start=(p == 0),
                        stop=(p == 3 and npass == 1),
                        perf_mode=DR,
                        skip_group_check=True,
                    )
            if USE_R8:
                for p in range(4):
                    for h in range(2):
                        nc.tensor.matmul(
                            psum[:, h * 512 : (h + 1) * 512],
                            q[:, 2 * p : 2 * p + 2, :],
                            etr_sb[:, 2 * p : 2 * p + 2, h * 512 : (h + 1) * 512],
                            start=False,
                            stop=(p == 3),
                            perf_mode=DR,
                            skip_group_check=True,
                        )
            s["psum"] = psum

        def back(S, r):
            s = st[S]
            eu = s["eu"]
            qhat = qpool.tile([128, N], f8, tag=f"qh{S}")
            n_ap = nbuf[:, S, r : r + 1]
            nh = smalls.tile([128, 2], f32, tag=f"nh{S}")
            for h in range(2):
                cs = slice(h * 512, (h + 1) * 512)
                nc.vector.scalar_tensor_tensor(
                    out=qhat[:, cs],
                    in0=s["psum"][:, cs],
                    scalar=s["rm"],
                    in1=eu[:, cs],
                    op0=ALU.mult,
                    op1=ALU.mult,
                    accum_out=nh[:, h : h + 1],
                )
            nc.vector.tensor_add(n_ap, nh[:, 0:1], nh[:, 1:2])
            rm = smalls.tile([128, 1], f32, tag=f"rm{S}")
            nc.vector.reciprocal(rm[:], n_ap)
            s["rm"] = rm[:]
            s["qhat"] = qhat
            if r == STEPS - 1:
                s["nlast"] = n_ap
                return
            qn = qnpool.tile([128, 8, B], f8, tag=f"qn{S}")
            for grp in range(2):
                tr = ps_tr.tile([128, 512], f32, tag="tr")
                for i4 in range(4):
                    it = grp * 4 + i4
                    nc.tensor.matmul(
                        tr[:, i4 * 128 : (i4 + 1) * 128],
                        qhat[:, it * 128 : (it + 1) * 128],
                        ident[:],
                        start=True,
                        stop=True,
                    )
                nc.scalar.activation(qn[:, grp * 4 : (grp + 1) * 4, :], tr[:], AF.Copy)
            s["qcur"] = qn

        for r in range(STEPS):
            front(0, r)
            if r > 0:
                back(1, r - 1)
            front(1, r)
            back(0, r)
        back(1, STEPS - 1)

        # epilogue: csum = sum of ln(n) over owned steps; qfin = qhat/n
        for S in range(NSTREAM):
            lnt = consts.tile([128, L], f32, tag=f"ln{S}")
            nc.scalar.activation(lnt[:], nbuf[:, S, W:STEPS], AF.Ln)
            cs = smalls.tile([128, 1], f32, tag=f"cs{S}")
            nc.vector.tensor_reduce(cs[:], lnt[:], axis=mybir.AxisListType.X, op=ALU.add)
            nc.sync.dma_start(out=OUT_C[:, S : S + 1], in_=cs[:])

            rn = smalls.tile([128, 1], f32, tag=f"rn{S}")
            nc.vector.reciprocal(rn[:], st[S]["nlast"])
            qf = consts.tile([128, N], f32, tag=f"qf{S}")
            nc.scalar.activation(qf[:], st[S]["qhat"][:], AF.Copy, scale=rn[:])
            nc.sync.dma_start(out=OUT_Q[S, :, :], in_=qf[:])

    nc.finalize()
    _dedupe_ldweights(nc)
    return nc


def _dedupe_ldweights(nc):
    """Remove back-to-back duplicate Ldweights (same memref/offset/ap/perf_mode
    separated only by Matmults). bacc emits one Ldweights per matmul; our
    DoubleRow pairs issue two matmuls (the two 512-col PSUM halves) from the
    same stationary, so every second load is redundant (~213 ns each on the
    PE weight path). Only sync-free duplicates are dropped, so the semaphore
    graph is unchanged."""
    removed = 0
    for fn in nc.m.functions:
        for blk in fn.blocks:
            out = []
            last_key = None
            for ins in blk.instructions:
                if ins.opcode == "Ldweights":
                    w = ins.ins[0]
                    key = (w.memref, w.offset, str(w.ap),
                           str(getattr(ins, "perf_mode", None)))
                    si = ins.sync_info
                    clean = si is None or (not si.on_wait and not si.on_update)
                    if clean and key == last_key:
                        removed += 1
                        continue
                    last_key = key
                elif ins.opcode != "Matmult":
                    last_key = None
                out.append(ins)
            blk.instructions = out
    return removed


def _get_nc():
    if "nc" not in _compiled:
        _compiled["nc"] = _build_bass()
    return _compiled["nc"]


def _prep_inputs(unary, transitions, start_idx):
    """Host-side: bf16 cast + per-core/stream halo gather into [NSTREAM, B, STEPS, N]."""
    unary = np.asarray(unary, dtype=np.float32)
    transitions = np.asarray(transitions, dtype=np.float32)

    fake = np.full((W, N), -14.0, dtype=np.float32)
    fake[:, start_idx] = 0.0
    g = np.exp(np.concatenate([fake, unary - np.float32(SHIFT)], axis=0)).astype(_BF)

    e32 = np.exp(transitions).T  # [j, i] = E[i, j]
    et8 = np.ascontiguousarray(e32).astype(_F8)
    maps_extra = {}
    if USE_R8:
        etr = (e32 - et8.astype(np.float32)).astype(_F8)
        maps_extra["etr"] = etr

    row_bytes = N * 2
    in_maps = []
    for c in range(NCORES):
        views = []
        for S in range(NSTREAM):
            base = g[c * PERCORE + S * PERSTREAM :]
            v = np.lib.stride_tricks.as_strided(
                base, shape=(B, STEPS, N), strides=(L * row_bytes, row_bytes, 2)
            )
            views.append(v)
        u_c = np.ascontiguousarray(np.stack(views, axis=0))
        in_maps.append({"u": u_c, "et": et8, **maps_extra})
    return in_maps


def _combine(results, transitions, end_idx):
    transitions = np.asarray(transitions, dtype=np.float64)
    total = 0.0
    for r in results:
        total += float(r["csum"].astype(np.float64).sum())
    total += SHIFT * T
    q_T = results[-1]["qfin"][NSTREAM - 1, B - 1].astype(np.float64)
    tau = np.exp(transitions[end_idx])
    total += float(np.log(np.dot(tau, q_T)))
    return total


def kernel(unary, transitions, start_idx, end_idx, _trace=False):
    from concourse.bass_utils import run_bass_kernel_spmd

    start_idx = int(np.asarray(start_idx))
    end_idx = int(np.asarray(end_idx))

    nc = _get_nc()
    in_maps = _prep_inputs(unary, transitions, start_idx)
    res = run_bass_kernel_spmd(nc, in_maps, core_ids=list(range(NCORES)), trace=_trace)
    _compiled["last_result"] = res
    logZ = _combine(res.results, transitions, end_idx)
    return np.array(logZ, dtype=np.float32)


# revision 14
# speedup vs baseline: 1.0673x; 1.0192x over previous
"""CRF forward (log-partition) kernel for Trainium2, 8 NeuronCores.

Algorithm: the forward recurrence in rescaled linear space is
    p_{t+1} = diag(exp(u_t)) @ E @ p_t,   E = exp(transitions)
(contraction over j = second index of transitions). Products of positive
matrices are strong Hilbert-projective contractions (~2 orders of
magnitude per step here), so the normalized direction of p_t forgets its
initial condition within a handful of steps. This cuts the T=65536-step
chain into C=2048 chunks of L=32 steps, each re-derived independently
from an arbitrary start vector with a W=6-step warm-up halo, with NO
cross-chunk communication:

    logZ = sum_chunks sum_owned ln(n_s) + SHIFT*T + ln(tau . qhat_T)

Per core: 256 chunks as TWO independent interleaved streams of B=128
chunk-columns (stream Y phase-shifted half a round behind X) so the
per-step dependency chain of one stream (matmul -> fused mul+norm ->
transpose -> PSUM evict) hides under the other stream's engine work.

Per stream step:
  - 8 DoubleRow fp8(e4m3) matmuls (K=256 each) compute psum[b,i] =
    sum_j q[j,b] * E^T[j,i] (stationary = state jt-pair, each LDWEIGHTS
    feeds the two 512-column halves).
  - one DVE scalar_tensor_tensor: qhat = (psum * 1/n_prev) * exp(u_row)
    with fused free-dim sum -> n_s (the per-column L1 normalizer,
    recorded in nbuf; exact per-step normalization keeps the fp8 state
    in range). exp(u) is batch-precomputed on ScalarE per DMA group
    with u pre-shifted by -(8 - ln 32) on host so n_s stays ~32.
  - 8 transpose-matmuls against an fp8 identity (2 groups of 4) +
    ScalarE PSUM->SBUF evictions produce the next stationary [j,b] pairs.

ln(n) slots are batch-computed at the end (one Ln + reduce per stream).
Chunk 0 of core 0 uses fake unary rows (0 at start_idx, -14 else)
which force q to the exact one-hot start vector within a few steps.
"""

import math
import numpy as np
import ml_dtypes
from contextlib import ExitStack

T = 65536
N = 1024
NCORES = 8
B = 128           # chunk-columns per stream (matmul M dim)
NSTREAM = 2
L = 32            # chunk length (owned steps)
W = 2             # warm-up halo steps
STEPS = W + L     # 34
PERCORE = T // NCORES          # 8192
PERSTREAM = PERCORE // NSTREAM  # 4096
SHIFT = float(np.float32(8.0 - math.log(32.0)))  # u preshift; added back on host

USE_R8 = False    # error-feedback residual matmul pass (accuracy fallback)

_BF = ml_dtypes.bfloat16
_F8 = ml_dtypes.float8_e4m3

_compiled = {}


def _build_bass():
    import concourse.bacc as bacc
    import concourse.tile as tile
    from concourse import mybir
    from concourse.masks import make_identity

    bf = mybir.dt.bfloat16
    f8 = mybir.dt.float8e4
    f32 = mybir.dt.float32
    AF = mybir.ActivationFunctionType
    ALU = mybir.AluOpType
    DR = mybir.MatmulPerfMode.DoubleRow

    nc = bacc.Bacc("TRN2", name="crf_fwd2")

    U = nc.dram_tensor("u", [NSTREAM, B, STEPS, N], bf, kind="ExternalInput")
    ET8 = nc.dram_tensor("et", [N, N], f8, kind="ExternalInput")
    if USE_R8:
        ETR = nc.dram_tensor("etr", [N, N], f8, kind="ExternalInput")
    OUT_C = nc.dram_tensor("csum", [B, NSTREAM], f32, kind="ExternalOutput")
    OUT_Q = nc.dram_tensor("qfin", [NSTREAM, B, N], f32, kind="ExternalOutput")

    with tile.TileContext(nc) as tc, ExitStack() as ctx:
        consts = ctx.enter_context(tc.tile_pool(name="consts", bufs=1))
        upool = ctx.enter_context(tc.tile_pool(name="u", bufs=3))
        qpool = ctx.enter_context(tc.tile_pool(name="qhat", bufs=3))
        qnpool = ctx.enter_context(tc.tile_pool(name="qn", bufs=4))
        smalls = ctx.enter_context(tc.tile_pool(name="smalls", bufs=6))
        ps_mm = ctx.enter_context(tc.tile_pool(name="psmm", bufs=2, space="PSUM"))
        ps_tr = ctx.enter_context(tc.tile_pool(name="pstr", bufs=2, space="PSUM"))

        ident = consts.tile([128, 128], f8)
        make_identity(nc, ident)

        # et_sb[j, jt, i] = E^T[jt*128+j, i] = E[i, jt*128+j]
        et_sb = consts.tile([128, 8, N], f8)
        nc.sync.dma_start(out=et_sb[:], in_=ET8.ap().rearrange("(jt j) i -> j jt i", j=128))
        if USE_R8:
            etr_sb = consts.tile([128, 8, N], f8)
            nc.sync.dma_start(
                out=etr_sb[:], in_=ETR.ap().rearrange("(jt j) i -> j jt i", j=128)
            )

        # per-(column, step) L1 normalizers, one slot each (Ln'd at the end)
        nbuf = consts.tile([128, NSTREAM, STEPS], f32)
        nc.vector.memset(nbuf[:], 1.0)

        # initial state: uniform ones, layout [j_in_tile, jt, b]
        q_init = consts.tile([128, 8, B], f8)
        nc.vector.memset(q_init[:], 1.0 / 32.0)
        rm_init = consts.tile([128, NSTREAM], f32)
        nc.vector.memset(rm_init[:], 1.0)

        st = [
            {"qcur": q_init, "rm": rm_init[:, s : s + 1], "eu": None,
             "psum": None, "qhat": None, "nlast": None}
            for s in range(NSTREAM)
        ]

        def front(S, r):
            s = st[S]
            if r % 4 == 0:
                gl = min(4, STEPS - r)
                ut = upool.tile([128, gl, N], bf, tag=f"u{S}")
                nc.sync.dma_start(out=ut[:], in_=U[S, :, r : r + gl, :])
                s["ut"] = ut
            s["eu"] = s["ut"][:, r % 4, :]
            psum = ps_mm.tile([128, N], f32, tag="mm")
            q = s["qcur"]
            npass = 2 if USE_R8 else 1
            for p in range(4):
                for h in range(2):
                    nc.tensor.matmul(
                        psum[:, h * 512 : (h + 1) * 512],
                        q[:, 2 * p : 2 * p + 2, :],
                        et_sb[:, 2 * p : 2 * p + 2, h * 512 : (h + 1) * 512],
                        start=(p == 0),
                        stop=(p == 3 and npass == 1),
                        perf_mode=DR,
                        skip_group_check=True,
                    )
            if USE_R8:
                for p in range(4):
                    for h in range(2):
                        nc.tensor.matmul(
                            psum[:, h * 512 : (h + 1) * 512],
                            q[:, 2 * p : 2 * p + 2, :],
                            etr_sb[:, 2 * p : 2 * p + 2, h * 512 : (h + 1) * 512],
                            start=False,
                            stop=(p == 3),
                            perf_mode=DR,
                            skip_group_check=True,
                        )
            s["psum"] = psum

        def back(S, r):
            s = st[S]
            eu = s["eu"]
            qhat = qpool.tile([128, N], f8, tag=f"qh{S}")
            n_ap = nbuf[:, S, r : r + 1]
            nh = smalls.tile([128, 2], f32, tag=f"nh{S}")
            for h in range(2):
                cs = slice(h * 512, (h + 1) * 512)
                nc.vector.scalar_tensor_tensor(
                    out=qhat[:, cs],
                    in0=s["psum"][:, cs],
                    scalar=s["rm"],
                    in1=eu[:, cs],
                    op0=ALU.mult,
                    op1=ALU.mult,
                    accum_out=nh[:, h : h + 1],
                )
            nc.vector.tensor_add(n_ap, nh[:, 0:1], nh[:, 1:2])
            rm = smalls.tile([128, 1], f32, tag=f"rm{S}")
            nc.vector.reciprocal(rm[:], n_ap)
            s["rm"] = rm[:]
            s["qhat"] = qhat
            if r == STEPS - 1:
                s["nlast"] = n_ap
                return
            qn = qnpool.tile([128, 8, B], f8, tag=f"qn{S}")
            for grp in range(2):
                tr = ps_tr.tile([128, 512], f32, tag="tr")
                for i4 in range(4):
                    it = grp * 4 + i4
                    nc.tensor.matmul(
                        tr[:, i4 * 128 : (i4 + 1) * 128],
                        qhat[:, it * 128 : (it + 1) * 128],
                        ident[:],
                        start=True,
                        stop=True,
                    )
                nc.scalar.activation(qn[:, grp * 4 : (grp + 1) * 4, :], tr[:], AF.Copy)
            s["qcur"] = qn

        for r in range(STEPS):
            front(0, r)
            if r > 0:
                back(1, r - 1)
            front(1, r)
            back(0, r)
        back(1, STEPS - 1)

        # epilogue: csum = sum of ln(n) over owned steps; qfin = qhat/n
        for S in range(NSTREAM):
            lnt = consts.tile([128, L], f32, tag=f"ln{S}")
            nc.scalar.activation(lnt[:], nbuf[:, S, W:STEPS], AF.Ln)
            cs = smalls.tile([128, 1], f32, tag=f"cs{S}")
            nc.vector.tensor_reduce(cs[:], lnt[:], axis=mybir.AxisListType.X, op=ALU.add)
            nc.sync.dma_start(out=OUT_C[:, S : S + 1], in_=cs[:])

            rn = smalls.tile([128, 1], f32, tag=f"rn{S}")
            nc.vector.reciprocal(rn[:], st[S]["nlast"])
            qf = consts.tile([128, N], f32, tag=f"qf{S}")
            nc.scalar.activation(qf[:], st[S]["qhat"][:], AF.Copy, scale=rn[:])
            nc.sync.dma_start(out=OUT_Q[S, :, :], in_=qf[:])

    nc.finalize()
    _dedupe_ldweights(nc)
    return nc


def _dedupe_ldweights(nc):
    """Remove back-to-back duplicate Ldweights (same memref/offset/ap/perf_mode
    separated only by Matmults). bacc emits one Ldweights per matmul; our
    DoubleRow pairs issue two matmuls (the two 512-col PSUM halves) from the
    same stationary, so every second load is redundant (~213 ns each on the
    PE weight path). Only sync-free duplicates are dropped, so the semaphore
    graph is unchanged."""
    removed = 0
    for fn in nc.m.functions:
        for blk in fn.blocks:
            out = []
            last_key = None
            for ins in blk.instructions:
                if ins.opcode == "Ldweights":
                    w = ins.ins[0]
                    key = (w.memref, w.offset, str(w.ap),
                           str(getattr(ins, "perf_mode", None)))
                    si = ins.sync_info
                    clean = si is None or (not si.on_wait and not si.on_update)
                    if clean and key == last_key:
                        removed += 1
                        continue
                    last_key = key
                elif ins.opcode != "Matmult":
                    last_key = None
                out.append(ins)
            blk.instructions = out
    return removed


def _get_nc():
    if "nc" not in _compiled:
        _compiled["nc"] = _build_bass()
    return _compiled["nc"]


def _prep_inputs(unary, transitions, start_idx):
    """Host-side: bf16 cast + per-core/stream halo gather into [NSTREAM, B, STEPS, N]."""
    unary = np.asarray(unary, dtype=np.float32)
    transitions = np.asarray(transitions, dtype=np.float32)

    fake = np.full((W, N), -14.0, dtype=np.float32)
    fake[:, start_idx] = 0.0
    g = np.exp(np.concatenate([fake, unary - np.float32(SHIFT)], axis=0)).astype(_BF)

    e32 = np.exp(transitions).T  # [j, i] = E[i, j]
    et8 = np.ascontiguousarray(e32).astype(_F8)
    maps_extra = {}
    if USE_R8:
        etr = (e32 - et8.astype(np.float32)).astype(_F8)
        maps_extra["etr"] = etr

    row_bytes = N * 2
    in_maps = []
    for c in range(NCORES):
        views = []
        for S in range(NSTREAM):
            base = g[c * PERCORE + S * PERSTREAM :]
            v = np.lib.stride_tricks.as_strided(
                base, shape=(B, STEPS, N), strides=(L * row_bytes, row_bytes, 2)
            )
            views.append(v)
        u_c = np.ascontiguousarray(np.stack(views, axis=0))
        in_maps.append({"u": u_c, "et": et8, **maps_extra})
    return in_maps


def _combine(results, transitions, end_idx):
    transitions = np.asarray(transitions, dtype=np.float64)
    total = 0.0
    for r in results:
        total += float(r["csum"].astype(np.float64).sum())
    total += SHIFT * T
    q_T = results[-1]["qfin"][NSTREAM - 1, B - 1].astype(np.float64)
    tau = np.exp(transitions[end_idx])
    total += float(np.log(np.dot(tau, q_T)))
    return total


def kernel(unary, transitions, start_idx, end_idx, _trace=False):
    from concourse.bass_utils import run_bass_kernel_spmd

    start_idx = int(np.asarray(start_idx))
    end_idx = int(np.asarray(end_idx))

    nc = _get_nc()
    in_maps = _prep_inputs(unary, transitions, start_idx)
    res = run_bass_kernel_spmd(nc, in_maps, core_ids=list(range(NCORES)), trace=_trace)
    _compiled["last_result"] = res
    logZ = _combine(res.results, transitions, end_idx)
    return np.array(logZ, dtype=np.float32)
